# revision 13
# baseline (speedup 1.0000x reference)
"""Classical self-attention block (QKV proj -> softmax attention -> out proj
-> residual + LayerNorm) on 8 Trainium2 NeuronCores.

Sharding: sequence-parallel over queries. Core c handles batch c//4, query
rows (c%4)*1024 .. +1024. Each core recomputes K/V for its whole batch
(no collectives). The per-batch input is rolled on the host so the core's
query rows are always rows 0..1023 -- softmax over keys is permutation
invariant, so attention output for those queries is unchanged.

v2 over the baseline (429us -> 367us measured on HW):
  - exp is split between ScalarE (native Exp, ~3/4 of tiles) and DVE
    (one-pass Schraudolph: u = A*s + (B + 1.5*2^23); the fp32 RNE add
    quantizes u to an integer whose low 16 bits are exactly the bf16 bit
    pattern of ~e^(s-12), fed to the PV matmul via a stride-2 bf16 view
    of the fp32 tile). Takes ScalarE off the critical path (218us busy).
  - softmax normalize: DVE copy of the PSUM denominator row ->
    reciprocal_approx_fast (must NOT read PSUM directly: computes garbage
    on HW although CoreSim accepts it) -> gpsimd partition-broadcast ->
    one scalar_tensor_tensor (pv * rden) straight from PSUM into at_t,
    DEFERRED past the next iteration's exp so the DVE queue never
    head-of-line-blocks the round boundary (was 6.5us/round of PE stall).
  - all PSUM->SBUF copies and LN tails are deferred the same way
    (pending_dve), keeping exp latency low; ATT_LAG=5 covers the
    normalize chain. GpSimd instructions cannot access PSUM (walrus).
  - input DMA ordered wk -> xt[cols 0:512] -> ... so the first projection
    matmul starts at ~12us instead of ~22us.
  - PE (TensorMatrix) is the bottleneck at ~306us busy / 82% occupancy;
    scores+PV at bf16 are 2x512 cols/iteration and already use the full
    128-row array. fp8 DoubleRow was measured to give ZERO net gain (2
    rhs elems/cycle but 2 contraction planes per logical column = same
    MAC rate as bf16, and the M<=64 ISA limit kills the ones-column
    denominator trick); DoublePixel/DoubleColumn are silently ignored by
    walrus codegen (identical timing and results).
"""

import numpy as np
import ml_dtypes

import concourse.bass as bass
import concourse.mybir as mybir
import concourse.tile as tile
from concourse import bacc
from concourse.bass_utils import run_bass_kernel_spmd

B, S, D = 2, 4096, 512
H, Dh = 8, 64
SQ = 1024            # query rows per core
SCALE = 1.0 / np.sqrt(Dh)
SHIFT = 12.0         # constant exp shift; cancels exactly in softmax
LN_EPS = 1e-5
N_CORES = 8

F32 = mybir.dt.float32
BF16 = mybir.dt.bfloat16

DC = D // 128        # 4 d-chunks (contraction for projections)
EC = D // 128        # 4 e-chunks (output chunks of projections)
TC = S // 128        # 32 t-chunks (keys)
TB = S // 512        # 8 t-blocks of 512
QB = SQ // 512       # 2 query blocks of 512
QC = SQ // 128       # 8 query chunks of 128

# Schraudolph constants (c=7 tuned for zero-mean rel err; see numerics.py)
A_SCH = float(np.float32(128.0 * np.log2(np.e)))
B_SCH = float(np.float32(128.0 * (127.0 - SHIFT * np.log2(np.e)) - 7.0
                         + 1.5 * 2.0**23))

ATT_LAG = 5          # scores+exp run LAG iterations ahead of PV


def dve_route(g):
    # DVE exp share: light during copy-heavy rounds 0-2, 1/4 after
    if g < 3 * TC:
        return g % 8 == 2
    return g % 4 == 2


def build_nc(unit_ln=False):
    nc = bacc.Bacc("TRN2", target_bir_lowering=False, debug=False,
                   num_devices=N_CORES)

    xt = nc.dram_tensor("xt", [D, S], BF16, kind="ExternalInput")
    xq = nc.dram_tensor("xq", [SQ, D], F32, kind="ExternalInput")
    wqt = nc.dram_tensor("wqt", [D, D], BF16, kind="ExternalInput")
    wkt = nc.dram_tensor("wkt", [D, D], BF16, kind="ExternalInput")
    wvt = nc.dram_tensor("wvt", [D, D], BF16, kind="ExternalInput")
    wot = nc.dram_tensor("wot", [D, D], BF16, kind="ExternalInput")
    gamma = nc.dram_tensor("gamma", [D], F32, kind="ExternalInput")
    beta = nc.dram_tensor("beta", [D], F32, kind="ExternalInput")
    out = nc.dram_tensor("out", [SQ, D], F32, kind="ExternalOutput")

    with tile.TileContext(nc) as tc:
        with (
            tc.tile_pool(name="const", bufs=1) as p_const,
            tc.tile_pool(name="wts", bufs=1) as p_w,
            tc.tile_pool(name="xtp", bufs=1) as p_xt,
            tc.tile_pool(name="kt", bufs=1) as p_kt,
            tc.tile_pool(name="vv", bufs=1) as p_v,
            tc.tile_pool(name="qt", bufs=1) as p_qt,
            tc.tile_pool(name="at", bufs=1) as p_at,
            tc.tile_pool(name="xqp", bufs=1) as p_xq,
            tc.tile_pool(name="ee", bufs=6) as p_e,
            tc.tile_pool(name="uu", bufs=3) as p_u,
            tc.tile_pool(name="nrm", bufs=2) as p_nrm,
            tc.tile_pool(name="ln", bufs=2) as p_ln,
            # PSUM: 2x[128,1024] score slots + 2x[128,512] proj slots
            # + 2 pv accumulators = 8 banks exactly
            tc.tile_pool(name="psA", bufs=2, space="PSUM") as ps_a,
            tc.tile_pool(name="psPV", bufs=1, space="PSUM") as ps_pv,
        ):
            # ---- inputs, ordered so the first kt matmul starts early ----
            w_tiles = {}

            def load_w(name, handle):
                t = p_w.tile([128, DC, D], BF16, tag=name, name=name)
                nc.sync.dma_start(
                    out=t, in_=handle.ap().rearrange("(c p) e -> p c e", p=128))
                w_tiles[name] = t

            load_w("wk", wkt)
            # xt split [0:512 | 512:2048 | 2048:4096] per d-chunk; separate
            # tiles so readers only depend on the piece they use, and the
            # first kt matmul starts after ~1MB of DMA
            xt_t = [[None] * 3 for _ in range(DC)]
            XSPLIT = (0, 512, 2048, 4096)
            for piece in range(3):
                if piece == 2:
                    load_w("wv", wvt)
                c0, c1 = XSPLIT[piece], XSPLIT[piece + 1]
                for dc in range(DC):
                    t = p_xt.tile([128, c1 - c0], BF16,
                                  tag=f"xt{dc}_{piece}",
                                  name=f"xt{dc}_{piece}")
                    nc.sync.dma_start(
                        out=t, in_=xt[dc * 128:(dc + 1) * 128, c0:c1])
                    xt_t[dc][piece] = t
            load_w("wq", wqt)
            xq_t = p_xq.tile([128, QC, D], F32, tag="xq")
            nc.sync.dma_start(
                out=xq_t, in_=xq.ap().rearrange("(n p) e -> p n e", p=128))
            wot_t = p_const.tile([128, EC, D], BF16, tag="wo")
            nc.sync.dma_start(
                out=wot_t, in_=wot.ap().rearrange("(c p) e -> p c e", p=128))
            gamma_b = p_const.tile([128, D], F32, tag="gamma_b")
            beta_b = p_const.tile([128, D], F32, tag="beta_b")
            nc.sync.dma_start(
                out=gamma_b,
                in_=bass.AP(tensor=gamma, offset=0, ap=[[0, 128], [1, D]]))
            nc.sync.dma_start(
                out=beta_b,
                in_=bass.AP(tensor=beta, offset=0, ap=[[0, 128], [1, D]]))
            eps_t = p_const.tile([128, 1], F32, tag="eps")
            nc.vector.memset(eps_t, LN_EPS)
            nshift_t = p_const.tile([128, 1], F32, tag="nshift")
            nc.vector.memset(nshift_t, -SHIFT)

            def xt_cols(dc, c0, c1):
                # columns c0:c1 of logical xt[dc] (must stay in one piece)
                for piece in range(3):
                    p0, p1 = XSPLIT[piece], XSPLIT[piece + 1]
                    if c0 >= p0 and c1 <= p1:
                        return xt_t[dc][piece][:, c0 - p0:c1 - p0]
                raise AssertionError((dc, c0, c1))

            # ---- persistent activations ----
            kt_t = p_kt.tile([128, EC, S], BF16, tag="kt")       # K^T [e, t]
            qt_t = p_qt.tile([128, EC, SQ], BF16, tag="qt")      # Q^T [e, s]
            # V with a ones column per head slot ([V(64) | 1]): the PV
            # matmul emits the softmax denominator as output row 64 for
            # free. Padded so each head's PV lhsT reads 128 columns.
            v_t = p_v.tile([128, TC, H * 65 + 64], BF16, tag="v")
            at_t = p_at.tile([128, EC, SQ], BF16, tag="at")      # A^T [e', s]

            nc.vector.memset(v_t[:, :, H * 65:], 0.0)  # keep pad finite
            ones_cols = v_t[:, :, 0:H * 65].rearrange(
                "p a (h x) -> p a h x", x=65)[:, :, :, 64:65]
            nc.vector.memset(ones_cols, 1.0)

            # ---------- emission helpers (program order = emit order) ------
            # Deferred DVE work (PSUM->SBUF copies, LN tails): flushed after
            # the iteration's exp so the exp never queues behind them.
            pending_dve = []

            def flush_dve():
                while pending_dve:
                    pending_dve.pop(0)()

            def emit_kt_group(ec, tb):
                # K^T[e, t] = sum_d wkt[d, e] * xt[d, t]
                ps = ps_a.tile([128, 512], F32, tag="small", name="psk")
                for dc in range(DC):
                    nc.tensor.matmul(
                        ps,
                        w_tiles["wk"][:, dc, ec * 128:(ec + 1) * 128],
                        xt_cols(dc, tb * 512, (tb + 1) * 512),
                        start=(dc == 0), stop=(dc == DC - 1))
                pending_dve.append(lambda ps=ps: nc.vector.tensor_copy(
                    out=kt_t[:, ec, tb * 512:(tb + 1) * 512], in_=ps))

            def emit_qt_group(ec, qb):
                # Q^T[e, s] = sum_d wqt[d, e] * xt[d, s]  (s < 1024)
                ps = ps_a.tile([128, 512], F32, tag="small", name="psq")
                for dc in range(DC):
                    nc.tensor.matmul(
                        ps,
                        w_tiles["wq"][:, dc, ec * 128:(ec + 1) * 128],
                        xt_cols(dc, qb * 512, (qb + 1) * 512),
                        start=(dc == 0), stop=(dc == DC - 1))
                pending_dve.append(lambda ps=ps: nc.vector.tensor_copy(
                    out=qt_t[:, ec, qb * 512:(qb + 1) * 512], in_=ps))

            def emit_ktqt(ec):
                for tb in range(TB):
                    emit_kt_group(ec, tb)
                    flush_dve()
                for qb in range(QB):
                    emit_qt_group(ec, qb)
                    flush_dve()

            def emit_v(tcb):
                # V[t, e] = sum_d xt[d, t] * wvt[d, e]
                ps = ps_a.tile([128, 512], F32, tag="small", name="psv")
                for dc in range(DC):
                    nc.tensor.matmul(
                        ps,
                        xt_cols(dc, tcb * 128, (tcb + 1) * 128),
                        w_tiles["wv"][:, dc, :],
                        start=(dc == 0), stop=(dc == DC - 1))
                v_dst = v_t[:, tcb, 0:H * 65].rearrange(
                    "p (h x) -> p h x", x=65)[:, :, 0:64]
                pending_dve.append(lambda ps=ps: nc.vector.tensor_copy(
                    out=v_dst, in_=ps.rearrange("p (h x) -> p h x", x=64)))

            def emit_normalize(j, qb, pv):
                # at[e', s] = pv[0:64] / pv[64]: fast approx reciprocal of
                # the PSUM denominator row, DMA partition-broadcast (stays
                # off the DVE/gpsimd queues), then one deferred fused
                # multiply-copy from PSUM into at_t.
                for i in range(2):
                    dd = p_nrm.tile([1, 512], F32, tag=f"dd{i}",
                                    name=f"dd{i}")
                    nc.vector.tensor_copy(out=dd, in_=pv[i][64:65, :])
                    rr = p_nrm.tile([1, 512], F32, tag=f"rr{i}",
                                    name=f"rr{i}")
                    nc.vector.reciprocal_approx_fast(out=rr, in_=dd)
                    rbc = p_nrm.tile([64, 512], F32, tag=f"rb{i}",
                                     name=f"rb{i}")
                    nc.gpsimd.partition_broadcast(rbc, rr, channels=64)
                    lo = i * 64

                    def _stt(i=i, j=j, qb=qb, rbc=rbc, lo=lo):
                        nc.vector.scalar_tensor_tensor(
                            out=at_t[lo:lo + 64, j,
                                     qb * 512:(qb + 1) * 512],
                            in0=pv[i][0:64, :], scalar=0.0, in1=rbc,
                            op0=mybir.AluOpType.bypass,
                            op1=mybir.AluOpType.mult)
                    pending_dve.append(_stt)

            def emit_ln(sc8):
                # Y2[s, e] = sum_e' at[e', s] * wot[e', e]; z = Y2 + xq
                ps = ps_a.tile([128, 512], F32, tag="small", name="pso")
                for ecp in range(EC):
                    nc.tensor.matmul(
                        ps,
                        at_t[:, ecp, sc8 * 128:(sc8 + 1) * 128],
                        wot_t[:, ecp, :],
                        start=(ecp == 0), stop=(ecp == EC - 1))
                pending_dve.append(lambda ps=ps: emit_ln_tail(sc8, ps))

            def emit_ln_tail(sc8, ps):
                z = p_ln.tile([128, D], F32, tag="z", name="z")
                nc.vector.tensor_add(z, ps, xq_t[:, sc8, :])
                stats = p_ln.tile([128, 6], F32, tag="stats", name="stats")
                nc.vector.bn_stats(out=stats, in_=z)
                mv = p_ln.tile([128, 2], F32, tag="mv", name="mv")
                nc.vector.bn_aggr(out=mv, in_=stats)
                std = p_ln.tile([128, 1], F32, tag="std", name="std")
                nc.scalar.activation(
                    out=std, in_=mv[:, 1:2],
                    func=mybir.ActivationFunctionType.Sqrt,
                    bias=eps_t, scale=1.0)
                rstd = p_ln.tile([128, 1], F32, tag="rstd", name="rstd")
                nc.vector.reciprocal(out=rstd, in_=std)
                nc.vector.tensor_scalar(
                    out=z, in0=z, scalar1=mv[:, 0:1], scalar2=rstd,
                    op0=mybir.AluOpType.subtract, op1=mybir.AluOpType.mult)
                if not unit_ln:
                    nc.vector.tensor_mul(z, z, gamma_b)
                    nc.vector.tensor_add(z, z, beta_b)
                nc.sync.dma_start(
                    out=out[sc8 * 128:(sc8 + 1) * 128, :], in_=z)

            def emit_attention_stream():
                # One continuous software-pipelined stream over all
                # (qb, j) rounds; PV trails by ATT_LAG iterations;
                # projection / V / LayerNorm work scheduled as PE fillers.
                rounds = [(qb, j) for qb in range(QB) for j in range(EC)]
                n_it = len(rounds) * TC
                fill = {}

                def add(g, th):
                    fill.setdefault(g, []).append(th)

                for t in range(TC):           # V chunks gate round-0 PVs
                    add(t, lambda t=t: emit_v(t))
                for r in (1, 2, 3):           # kt/qt for qb0 round r
                    base = (r - 1) * TC + 6   # mid-round, clear of edges
                    add(base, lambda r=r: emit_qt_group(r, 0))
                    for tb in range(TB):
                        add(base + 1 + tb,
                            lambda r=r, tb=tb: emit_kt_group(r, tb))
                    add(base + 9, lambda r=r: emit_qt_group(r, 1))
                add(3 * TC + 10, lambda: emit_qt_group(0, 1))
                for i, s in enumerate((4 * TC + 6, 4 * TC + 14,
                                       5 * TC + 6, 5 * TC + 14)):
                    add(s, lambda i=i: emit_ln(i))   # LN for qb0 chunks

                pv = [ps_pv.tile([128, 512], F32,
                                 tag=f"pv{i}", name=f"pv{i}")
                      for i in range(2)]
                ets = {}
                for g in range(n_it + ATT_LAG):
                    for th in fill.pop(g, ()):
                        th()
                    if g < n_it:
                        qb, j = rounds[g // TC]
                        u = g % TC
                        sc = ps_a.tile([128, 1024], F32, tag="sc",
                                       name="sc")
                        for i in range(2):
                            lo = i * 64
                            # scores^T head 2j+i -> bank-half i of sc
                            nc.tensor.matmul(
                                sc[:, i * 512:(i + 1) * 512],
                                kt_t[lo:lo + 64, j,
                                     u * 128:(u + 1) * 128],
                                qt_t[lo:lo + 64, j,
                                     qb * 512:(qb + 1) * 512],
                                start=True, stop=True,
                                tile_position=(lo, 0))
                        if dve_route(g):
                            ut = p_u.tile([128, 1024], F32, tag="u",
                                          name="ut")
                            nc.vector.tensor_scalar(
                                out=ut, in0=sc, scalar1=A_SCH,
                                scalar2=B_SCH,
                                op0=mybir.AluOpType.mult,
                                op1=mybir.AluOpType.add)
                            ub = ut.bitcast(BF16)
                            ets[g] = [ub[:, 0:1024:2], ub[:, 1024:2048:2]]
                        else:
                            et = p_e.tile([128, 1024], BF16, tag="e",
                                          name="et")
                            nc.scalar.activation(
                                out=et, in_=sc,
                                func=mybir.ActivationFunctionType.Exp,
                                bias=nshift_t, scale=1.0)
                            ets[g] = [et[:, 0:512], et[:, 512:1024]]
                    flush_dve()
                    gp = g - ATT_LAG
                    if gp >= 0:
                        qb_p, j_p = rounds[gp // TC]
                        u = gp % TC
                        rhs = ets.pop(gp)
                        for i in range(2):
                            h = 2 * j_p + i
                            nc.tensor.matmul(
                                pv[i],
                                v_t[:, u, h * 65:h * 65 + 128],
                                rhs[i],
                                start=(u == 0), stop=(u == TC - 1))
                        if u == TC - 1:
                            emit_normalize(j_p, qb_p, pv)
                flush_dve()

            # ---------- program order ----------
            emit_ktqt(0)
            emit_attention_stream()
            for s in range(4, 8):
                emit_ln(s)
                flush_dve()

    nc.finalize()
    return nc


_NC = None
_NC_KIND = None


def kernel(rotation_params=None, entangle_params=None, inputs=None,
           Wq=None, Wk=None, Wv=None, Wo=None, ln_gamma=None, ln_beta=None,
           _trace=False, **_unused):
    global _NC
    X = np.ascontiguousarray(np.asarray(inputs, np.float32))
    Wq = np.asarray(Wq, np.float32)
    Wk = np.asarray(Wk, np.float32)
    Wv = np.asarray(Wv, np.float32)
    Wo = np.asarray(Wo, np.float32)
    g = np.ascontiguousarray(np.asarray(ln_gamma, np.float32))
    b = np.ascontiguousarray(np.asarray(ln_beta, np.float32))

    wqt = np.ascontiguousarray(Wq.T * SCALE).astype(ml_dtypes.bfloat16)
    wkt = np.ascontiguousarray(Wk.T).astype(ml_dtypes.bfloat16)
    wvt = np.ascontiguousarray(Wv.T).astype(ml_dtypes.bfloat16)
    wot = np.ascontiguousarray(Wo.T).astype(ml_dtypes.bfloat16)

    in_maps = []
    for c in range(N_CORES):
        bb, q0 = c // 4, (c % 4) * SQ
        Xb = np.roll(X[bb], -q0, axis=0)
        in_maps.append({
            "xt": np.ascontiguousarray(Xb.T).astype(ml_dtypes.bfloat16),
            "xq": np.ascontiguousarray(Xb[:SQ]),
            "wqt": wqt, "wkt": wkt, "wvt": wvt, "wot": wot,
            "gamma": g, "beta": b,
        })

    unit_ln = bool(np.all(g == 1.0) and np.all(b == 0.0))
    global _NC_KIND
    if _NC is None or _NC_KIND != unit_ln:
        _NC = build_nc(unit_ln=unit_ln)
        _NC_KIND = unit_ln

    res = run_bass_kernel_spmd(_NC, in_maps, core_ids=list(range(N_CORES)),
                               trace=_trace)
    out = np.empty((B, S, D), np.float32)
    for c in range(N_CORES):
        bb, q0 = c // 4, (c % 4) * SQ
        out[bb, q0:q0 + SQ] = res.results[c]["out"]
    if _trace:
        kernel._last_results = res
    return out


# revision 14
# speedup vs baseline: 1.0018x; 1.0018x over previous
"""Classical self-attention block (QKV proj -> softmax attention -> out proj
-> residual + LayerNorm) on 8 Trainium2 NeuronCores.

Sharding: sequence-parallel over queries. Core c handles batch c//4, query
rows (c%4)*1024 .. +1024. Each core recomputes K/V for its whole batch
(no collectives). The per-batch input is rolled on the host so the core's
query rows are always rows 0..1023 -- softmax over keys is permutation
invariant, so attention output for those queries is unchanged.

v2 over the baseline (429us -> 367us measured on HW):
  - exp is split between ScalarE (native Exp, ~3/4 of tiles) and DVE
    (one-pass Schraudolph: u = A*s + (B + 1.5*2^23); the fp32 RNE add
    quantizes u to an integer whose low 16 bits are exactly the bf16 bit
    pattern of ~e^(s-12), fed to the PV matmul via a stride-2 bf16 view
    of the fp32 tile). Takes ScalarE off the critical path (218us busy).
  - softmax normalize: DVE copy of the PSUM denominator row ->
    reciprocal_approx_fast (must NOT read PSUM directly: computes garbage
    on HW although CoreSim accepts it) -> gpsimd partition-broadcast ->
    one scalar_tensor_tensor (pv * rden) straight from PSUM into at_t,
    DEFERRED past the next iteration's exp so the DVE queue never
    head-of-line-blocks the round boundary (was 6.5us/round of PE stall).
  - all PSUM->SBUF copies and LN tails are deferred the same way
    (pending_dve), keeping exp latency low; ATT_LAG=5 covers the
    normalize chain. GpSimd instructions cannot access PSUM (walrus).
  - input DMA ordered wk -> xt[cols 0:512] -> ... so the first projection
    matmul starts at ~12us instead of ~22us.
  - PE (TensorMatrix) is the bottleneck at ~306us busy / 82% occupancy;
    scores+PV at bf16 are 2x512 cols/iteration and already use the full
    128-row array. fp8 DoubleRow was measured to give ZERO net gain (2
    rhs elems/cycle but 2 contraction planes per logical column = same
    MAC rate as bf16, and the M<=64 ISA limit kills the ones-column
    denominator trick); DoublePixel/DoubleColumn are silently ignored by
    walrus codegen (identical timing and results).
"""

import numpy as np
import ml_dtypes

import concourse.bass as bass
import concourse.mybir as mybir
import concourse.tile as tile
from concourse import bacc
from concourse.bass_utils import run_bass_kernel_spmd

B, S, D = 2, 4096, 512
H, Dh = 8, 64
SQ = 1024            # query rows per core
SCALE = 1.0 / np.sqrt(Dh)
SHIFT = 12.0         # constant exp shift; cancels exactly in softmax
LN_EPS = 1e-5
N_CORES = 8

F32 = mybir.dt.float32
BF16 = mybir.dt.bfloat16

DC = D // 128        # 4 d-chunks (contraction for projections)
EC = D // 128        # 4 e-chunks (output chunks of projections)
TC = S // 128        # 32 t-chunks (keys)
TB = S // 512        # 8 t-blocks of 512
QB = SQ // 512       # 2 query blocks of 512
QC = SQ // 128       # 8 query chunks of 128

# Schraudolph constants (c=7 tuned for zero-mean rel err; see numerics.py)
A_SCH = float(np.float32(128.0 * np.log2(np.e)))
B_SCH = float(np.float32(128.0 * (127.0 - SHIFT * np.log2(np.e)) - 7.0
                         + 1.5 * 2.0**23))

ATT_LAG = 5          # scores+exp run LAG iterations ahead of PV


def dve_route(g):
    # DVE exp share: light during copy-heavy rounds 0-2, 1/4 after
    if g < 3 * TC:
        return g % 8 == 2
    return g % 4 == 2


def build_nc(unit_ln=False):
    nc = bacc.Bacc("TRN2", target_bir_lowering=False, debug=False,
                   num_devices=N_CORES)

    xt = nc.dram_tensor("xt", [D, S], BF16, kind="ExternalInput")
    xq = nc.dram_tensor("xq", [SQ, D], F32, kind="ExternalInput")
    wqt = nc.dram_tensor("wqt", [D, D], BF16, kind="ExternalInput")
    wkt = nc.dram_tensor("wkt", [D, D], BF16, kind="ExternalInput")
    wvt = nc.dram_tensor("wvt", [D, D], BF16, kind="ExternalInput")
    wot = nc.dram_tensor("wot", [D, D], BF16, kind="ExternalInput")
    gamma = nc.dram_tensor("gamma", [D], F32, kind="ExternalInput")
    beta = nc.dram_tensor("beta", [D], F32, kind="ExternalInput")
    out = nc.dram_tensor("out", [SQ, D], F32, kind="ExternalOutput")

    with tile.TileContext(nc) as tc:
        with (
            tc.tile_pool(name="const", bufs=1) as p_const,
            tc.tile_pool(name="wts", bufs=1) as p_w,
            tc.tile_pool(name="xtp", bufs=1) as p_xt,
            tc.tile_pool(name="kt", bufs=1) as p_kt,
            tc.tile_pool(name="vv", bufs=1) as p_v,
            tc.tile_pool(name="qt", bufs=1) as p_qt,
            tc.tile_pool(name="at", bufs=1) as p_at,
            tc.tile_pool(name="xqp", bufs=1) as p_xq,
            tc.tile_pool(name="ee", bufs=6) as p_e,
            tc.tile_pool(name="uu", bufs=3) as p_u,
            tc.tile_pool(name="nrm", bufs=2) as p_nrm,
            tc.tile_pool(name="ln", bufs=2) as p_ln,
            # PSUM: 2x[128,1024] score slots + 2x[128,512] proj slots
            # + 2 pv accumulators = 8 banks exactly
            tc.tile_pool(name="psA", bufs=2, space="PSUM") as ps_a,
            tc.tile_pool(name="psPV", bufs=1, space="PSUM") as ps_pv,
        ):
            # ---- inputs, ordered so the first kt matmul starts early ----
            w_tiles = {}

            def load_w(name, handle):
                t = p_w.tile([128, DC, D], BF16, tag=name, name=name)
                nc.sync.dma_start(
                    out=t, in_=handle.ap().rearrange("(c p) e -> p c e", p=128))
                w_tiles[name] = t

            # wk arrives per e-chunk: the whole first PE section
            # (emit_ktqt(0)) only reads e-chunk 0, so kt(0,0) starts after
            # ~640KB of DMA. xt is split [0:512 | 512:2048 | 2048:4096]
            # per d-chunk with separate tiles so readers only depend on
            # the piece they use.
            wk_l = []

            def load_wk_chunk(ec):
                t = p_w.tile([128, DC, 128], BF16, tag=f"wk{ec}",
                             name=f"wk{ec}")
                nc.sync.dma_start(
                    out=t,
                    in_=wkt[:, ec * 128:(ec + 1) * 128].rearrange(
                        "(c p) e -> p c e", p=128))
                wk_l.append(t)

            load_wk_chunk(0)
            xt_t = [[None] * 3 for _ in range(DC)]
            XSPLIT = (0, 512, 2048, 4096)
            for piece in range(3):
                if piece == 1:
                    load_w("wq", wqt)
                    load_w("wv", wvt)
                c0, c1 = XSPLIT[piece], XSPLIT[piece + 1]
                for dc in range(DC):
                    t = p_xt.tile([128, c1 - c0], BF16,
                                  tag=f"xt{dc}_{piece}",
                                  name=f"xt{dc}_{piece}")
                    nc.sync.dma_start(
                        out=t, in_=xt[dc * 128:(dc + 1) * 128, c0:c1])
                    xt_t[dc][piece] = t
            for ec in range(1, EC):
                load_wk_chunk(ec)
            xq_t = p_xq.tile([128, QC, D], F32, tag="xq")
            nc.sync.dma_start(
                out=xq_t, in_=xq.ap().rearrange("(n p) e -> p n e", p=128))
            wot_t = p_const.tile([128, EC, D], BF16, tag="wo")
            nc.sync.dma_start(
                out=wot_t, in_=wot.ap().rearrange("(c p) e -> p c e", p=128))
            gamma_b = p_const.tile([128, D], F32, tag="gamma_b")
            beta_b = p_const.tile([128, D], F32, tag="beta_b")
            nc.sync.dma_start(
                out=gamma_b,
                in_=bass.AP(tensor=gamma, offset=0, ap=[[0, 128], [1, D]]))
            nc.sync.dma_start(
                out=beta_b,
                in_=bass.AP(tensor=beta, offset=0, ap=[[0, 128], [1, D]]))
            eps_t = p_const.tile([128, 1], F32, tag="eps")
            nc.vector.memset(eps_t, LN_EPS)
            nshift_t = p_const.tile([128, 1], F32, tag="nshift")
            nc.vector.memset(nshift_t, -SHIFT)

            def xt_cols(dc, c0, c1):
                # columns c0:c1 of logical xt[dc] (must stay in one piece)
                for piece in range(3):
                    p0, p1 = XSPLIT[piece], XSPLIT[piece + 1]
                    if c0 >= p0 and c1 <= p1:
                        return xt_t[dc][piece][:, c0 - p0:c1 - p0]
                raise AssertionError((dc, c0, c1))

            # ---- persistent activations ----
            kt_t = p_kt.tile([128, EC, S], BF16, tag="kt")       # K^T [e, t]
            qt_t = p_qt.tile([128, EC, SQ], BF16, tag="qt")      # Q^T [e, s]
            # V with a ones column per head slot ([V(64) | 1]): the PV
            # matmul emits the softmax denominator as output row 64 for
            # free. Padded so each head's PV lhsT reads 128 columns.
            v_t = p_v.tile([128, TC, H * 65 + 64], BF16, tag="v")
            at_t = p_at.tile([128, EC, SQ], BF16, tag="at")      # A^T [e', s]

            nc.vector.memset(v_t[:, :, H * 65:], 0.0)  # keep pad finite
            ones_cols = v_t[:, :, 0:H * 65].rearrange(
                "p a (h x) -> p a h x", x=65)[:, :, :, 64:65]
            nc.vector.memset(ones_cols, 1.0)

            # ---------- emission helpers (program order = emit order) ------
            # Deferred DVE work (PSUM->SBUF copies, LN tails): flushed after
            # the iteration's exp so the exp never queues behind them.
            pending_dve = []

            def flush_dve():
                while pending_dve:
                    pending_dve.pop(0)()

            def emit_kt_group(ec, tb):
                # K^T[e, t] = sum_d wkt[d, e] * xt[d, t]
                ps = ps_a.tile([128, 512], F32, tag="small", name="psk")
                for dc in range(DC):
                    nc.tensor.matmul(
                        ps,
                        wk_l[ec][:, dc, :],
                        xt_cols(dc, tb * 512, (tb + 1) * 512),
                        start=(dc == 0), stop=(dc == DC - 1))
                pending_dve.append(lambda ps=ps: nc.vector.tensor_copy(
                    out=kt_t[:, ec, tb * 512:(tb + 1) * 512], in_=ps))

            def emit_qt_group(ec, qb):
                # Q^T[e, s] = sum_d wqt[d, e] * xt[d, s]  (s < 1024)
                ps = ps_a.tile([128, 512], F32, tag="small", name="psq")
                for dc in range(DC):
                    nc.tensor.matmul(
                        ps,
                        w_tiles["wq"][:, dc, ec * 128:(ec + 1) * 128],
                        xt_cols(dc, qb * 512, (qb + 1) * 512),
                        start=(dc == 0), stop=(dc == DC - 1))
                pending_dve.append(lambda ps=ps: nc.vector.tensor_copy(
                    out=qt_t[:, ec, qb * 512:(qb + 1) * 512], in_=ps))

            def emit_ktqt(ec):
                for tb in range(TB):
                    emit_kt_group(ec, tb)
                    flush_dve()
                for qb in range(QB):
                    emit_qt_group(ec, qb)
                    flush_dve()

            def emit_v(tcb):
                # V[t, e] = sum_d xt[d, t] * wvt[d, e]
                ps = ps_a.tile([128, 512], F32, tag="small", name="psv")
                for dc in range(DC):
                    nc.tensor.matmul(
                        ps,
                        xt_cols(dc, tcb * 128, (tcb + 1) * 128),
                        w_tiles["wv"][:, dc, :],
                        start=(dc == 0), stop=(dc == DC - 1))
                v_dst = v_t[:, tcb, 0:H * 65].rearrange(
                    "p (h x) -> p h x", x=65)[:, :, 0:64]
                pending_dve.append(lambda ps=ps: nc.vector.tensor_copy(
                    out=v_dst, in_=ps.rearrange("p (h x) -> p h x", x=64)))

            def emit_normalize(j, qb, pv):
                # at[e', s] = pv[0:64] / pv[64]: fast approx reciprocal of
                # the PSUM denominator row, DMA partition-broadcast (stays
                # off the DVE/gpsimd queues), then one deferred fused
                # multiply-copy from PSUM into at_t.
                for i in range(2):
                    dd = p_nrm.tile([1, 512], F32, tag=f"dd{i}",
                                    name=f"dd{i}")
                    nc.vector.tensor_copy(out=dd, in_=pv[i][64:65, :])
                    rr = p_nrm.tile([1, 512], F32, tag=f"rr{i}",
                                    name=f"rr{i}")
                    nc.vector.reciprocal_approx_fast(out=rr, in_=dd)
                    rbc = p_nrm.tile([64, 512], F32, tag=f"rb{i}",
                                     name=f"rb{i}")
                    nc.gpsimd.partition_broadcast(rbc, rr, channels=64)
                    lo = i * 64

                    def _stt(i=i, j=j, qb=qb, rbc=rbc, lo=lo):
                        nc.vector.scalar_tensor_tensor(
                            out=at_t[lo:lo + 64, j,
                                     qb * 512:(qb + 1) * 512],
                            in0=pv[i][0:64, :], scalar=0.0, in1=rbc,
                            op0=mybir.AluOpType.bypass,
                            op1=mybir.AluOpType.mult)
                    pending_dve.append(_stt)

            def emit_ln(sc8):
                # Y2[s, e] = sum_e' at[e', s] * wot[e', e]; z = Y2 + xq
                ps = ps_a.tile([128, 512], F32, tag="small", name="pso")
                for ecp in range(EC):
                    nc.tensor.matmul(
                        ps,
                        at_t[:, ecp, sc8 * 128:(sc8 + 1) * 128],
                        wot_t[:, ecp, :],
                        start=(ecp == 0), stop=(ecp == EC - 1))
                pending_dve.append(lambda ps=ps: emit_ln_tail(sc8, ps))

            def emit_ln_tail(sc8, ps):
                z = p_ln.tile([128, D], F32, tag="z", name="z")
                nc.vector.tensor_add(z, ps, xq_t[:, sc8, :])
                stats = p_ln.tile([128, 6], F32, tag="stats", name="stats")
                nc.vector.bn_stats(out=stats, in_=z)
                mv = p_ln.tile([128, 2], F32, tag="mv", name="mv")
                nc.vector.bn_aggr(out=mv, in_=stats)
                std = p_ln.tile([128, 1], F32, tag="std", name="std")
                nc.scalar.activation(
                    out=std, in_=mv[:, 1:2],
                    func=mybir.ActivationFunctionType.Sqrt,
                    bias=eps_t, scale=1.0)
                rstd = p_ln.tile([128, 1], F32, tag="rstd", name="rstd")
                nc.vector.reciprocal(out=rstd, in_=std)
                nc.vector.tensor_scalar(
                    out=z, in0=z, scalar1=mv[:, 0:1], scalar2=rstd,
                    op0=mybir.AluOpType.subtract, op1=mybir.AluOpType.mult)
                if not unit_ln:
                    nc.vector.tensor_mul(z, z, gamma_b)
                    nc.vector.tensor_add(z, z, beta_b)
                nc.sync.dma_start(
                    out=out[sc8 * 128:(sc8 + 1) * 128, :], in_=z)

            def emit_attention_stream():
                # One continuous software-pipelined stream over all
                # (qb, j) rounds; PV trails by ATT_LAG iterations;
                # projection / V / LayerNorm work scheduled as PE fillers.
                rounds = [(qb, j) for qb in range(QB) for j in range(EC)]
                n_it = len(rounds) * TC
                fill = {}

                def add(g, th):
                    fill.setdefault(g, []).append(th)

                for t in range(TC):           # V chunks gate round-0 PVs
                    add(t, lambda t=t: emit_v(t))
                for r in (1, 2, 3):           # kt/qt for qb0 round r
                    base = (r - 1) * TC + 6   # mid-round, clear of edges
                    add(base, lambda r=r: emit_qt_group(r, 0))
                    for tb in range(TB):
                        add(base + 1 + tb,
                            lambda r=r, tb=tb: emit_kt_group(r, tb))
                    add(base + 9, lambda r=r: emit_qt_group(r, 1))
                add(3 * TC + 10, lambda: emit_qt_group(0, 1))
                for i, s in enumerate((4 * TC + 6, 4 * TC + 14,
                                       5 * TC + 6, 5 * TC + 14)):
                    add(s, lambda i=i: emit_ln(i))   # LN for qb0 chunks

                pv = [ps_pv.tile([128, 512], F32,
                                 tag=f"pv{i}", name=f"pv{i}")
                      for i in range(2)]
                ets = {}
                for g in range(n_it + ATT_LAG):
                    for th in fill.pop(g, ()):
                        th()
                    if g < n_it:
                        qb, j = rounds[g // TC]
                        u = g % TC
                        sc = ps_a.tile([128, 1024], F32, tag="sc",
                                       name="sc")
                        for i in range(2):
                            lo = i * 64
                            # scores^T head 2j+i -> bank-half i of sc
                            nc.tensor.matmul(
                                sc[:, i * 512:(i + 1) * 512],
                                kt_t[lo:lo + 64, j,
                                     u * 128:(u + 1) * 128],
                                qt_t[lo:lo + 64, j,
                                     qb * 512:(qb + 1) * 512],
                                start=True, stop=True,
                                tile_position=(lo, 0))
                        if dve_route(g):
                            ut = p_u.tile([128, 1024], F32, tag="u",
                                          name="ut")
                            nc.vector.tensor_scalar(
                                out=ut, in0=sc, scalar1=A_SCH,
                                scalar2=B_SCH,
                                op0=mybir.AluOpType.mult,
                                op1=mybir.AluOpType.add)
                            ub = ut.bitcast(BF16)
                            ets[g] = [ub[:, 0:1024:2], ub[:, 1024:2048:2]]
                        else:
                            et = p_e.tile([128, 1024], BF16, tag="e",
                                          name="et")
                            nc.scalar.activation(
                                out=et, in_=sc,
                                func=mybir.ActivationFunctionType.Exp,
                                bias=nshift_t, scale=1.0)
                            ets[g] = [et[:, 0:512], et[:, 512:1024]]
                    flush_dve()
                    gp = g - ATT_LAG
                    if gp >= 0:
                        qb_p, j_p = rounds[gp // TC]
                        u = gp % TC
                        rhs = ets.pop(gp)
                        for i in range(2):
                            h = 2 * j_p + i
                            nc.tensor.matmul(
                                pv[i],
                                v_t[:, u, h * 65:h * 65 + 128],
                                rhs[i],
                                start=(u == 0), stop=(u == TC - 1))
                        if u == TC - 1:
                            emit_normalize(j_p, qb_p, pv)
                flush_dve()

            # ---------- program order ----------
            emit_ktqt(0)
            emit_attention_stream()
            for s in range(4, 8):
                emit_ln(s)
                flush_dve()

    nc.finalize()
    return nc


_NC = None
_NC_KIND = None


def kernel(rotation_params=None, entangle_params=None, inputs=None,
           Wq=None, Wk=None, Wv=None, Wo=None, ln_gamma=None, ln_beta=None,
           _trace=False, **_unused):
    global _NC
    X = np.ascontiguousarray(np.asarray(inputs, np.float32))
    Wq = np.asarray(Wq, np.float32)
    Wk = np.asarray(Wk, np.float32)
    Wv = np.asarray(Wv, np.float32)
    Wo = np.asarray(Wo, np.float32)
    g = np.ascontiguousarray(np.asarray(ln_gamma, np.float32))
    b = np.ascontiguousarray(np.asarray(ln_beta, np.float32))

    wqt = np.ascontiguousarray(Wq.T * SCALE).astype(ml_dtypes.bfloat16)
    wkt = np.ascontiguousarray(Wk.T).astype(ml_dtypes.bfloat16)
    wvt = np.ascontiguousarray(Wv.T).astype(ml_dtypes.bfloat16)
    wot = np.ascontiguousarray(Wo.T).astype(ml_dtypes.bfloat16)

    in_maps = []
    for c in range(N_CORES):
        bb, q0 = c // 4, (c % 4) * SQ
        Xb = np.roll(X[bb], -q0, axis=0)
        in_maps.append({
            "xt": np.ascontiguousarray(Xb.T).astype(ml_dtypes.bfloat16),
            "xq": np.ascontiguousarray(Xb[:SQ]),
            "wqt": wqt, "wkt": wkt, "wvt": wvt, "wot": wot,
            "gamma": g, "beta": b,
        })

    unit_ln = bool(np.all(g == 1.0) and np.all(b == 0.0))
    global _NC_KIND
    if _NC is None or _NC_KIND != unit_ln:
        _NC = build_nc(unit_ln=unit_ln)
        _NC_KIND = unit_ln

    res = run_bass_kernel_spmd(_NC, in_maps, core_ids=list(range(N_CORES)),
                               trace=_trace)
    out = np.empty((B, S, D), np.float32)
    for c in range(N_CORES):
        bb, q0 = c // 4, (c % 4) * SQ
        out[bb, q0:q0 + SQ] = res.results[c]["out"]
    if _trace:
        kernel._last_results = res
    return out
